# revision 56
# baseline (speedup 1.0000x reference)
"""Trainium2 Bass kernel for nn_AttentionLayer (attention pooling).

Reference computation (per sample b):
    scores[s, d] = tanh( sum_t X[t, d] * W[t, s] + bias[s] )   # X = inputs[b], [T=200, D=512]
    a = softmax over s of scores                                # per d column
    out[b, d] = sum_s a[s, d] * X[s, d]

Sharding: pure data parallel, batch 512 -> 64 samples on each of 8 cores.
W/b replicated. No collectives.

Layout (per sample): s on partitions (chunks 128+72), d on free axis; the
bias is folded into the score matmul via W_ext = [W; b] (host-side) and a
memset ones-row appended to X's second t-chunk. Each sample's scores live
in one [128, 1024] PSUM tile (s0-half cols 0:512 on 128 partitions,
s1-half cols 512:1024 on 72 partitions; unused lanes carry garbage that
tanh bounds and the reductions mask).

  - 4 float32r matmuls per sample accumulate scores into PSUM
  - one ACT tanh [128, 1024] per sample; exp in half-group chunks (bf16)
  - one DVE multiply P = E * X per 4-sample group [128, 4096]
  - bf16 ones-vector matmuls (col-tiled, 4 PE column groups) reduce over s
    into numer/denom rows at partitions {0,32,64,96}
  - batched DVE reciprocal+multiply epilogue, strided-row output DMA
"""

import sys

if "/opt/trn_rl_repo" not in sys.path:
    sys.path.insert(0, "/opt/trn_rl_repo")

import ml_dtypes
import numpy as np

import concourse.bass as bass
import concourse.tile as tile
from concourse import bacc, mybir
from concourse.bass_utils import run_bass_kernel_spmd

B, T, D = 512, 200, 512
N_CORES = 8
NS = B // N_CORES  # samples per core
T0 = 128           # first t/s chunk
T1 = T - T0        # 72
T1E = T1 + 1       # 73: second t-chunk + bias ones-row

F32 = mybir.dt.float32
F32R = mybir.dt.float32r
BF16 = mybir.dt.bfloat16

_CACHE = {}


def _build(ns=NS):
    nc = bacc.Bacc("TRN2", target_bir_lowering=False, debug=False)

    x_ext = nc.declare_dram_parameter("x", [ns, T + 1, D], F32R, isOutput=False)
    w_ext = nc.declare_dram_parameter("w", [T + 1, T], F32R, isOutput=False)
    onesb_ext = nc.declare_dram_parameter("onesb", [T0, 1], BF16, isOutput=False)
    out_ext = nc.declare_dram_parameter("out", [ns, D], F32, isOutput=True)

    with tile.TileContext(nc) as tc:
        with (
            tc.tile_pool(name="const", bufs=1) as cpool,
            tc.tile_pool(name="tanh", bufs=2) as tpool,
            tc.tile_pool(name="exp", bufs=3) as epool,
            tc.tile_pool(name="small", bufs=3) as apool,
            tc.tile_pool(name="xin", bufs=4) as xpool,
            tc.tile_pool(name="psum", bufs=2, space="PSUM") as ppool,
            tc.tile_pool(name="rpsum", bufs=2, space="PSUM") as rpool,
        ):
            # constants: extended weights ([W; b]), ones vector
            w0 = cpool.tile([T0, T], F32R)    # W[0:128, :]  (t on part, s free)
            w1 = cpool.tile([T1E, T], F32R)   # [W[128:200, :]; b]
            onesb = cpool.tile([T0, 1], BF16)


            nc.scalar.dma_start(w0[:], w_ext[0:T0, :])
            nc.scalar.dma_start(w1[:], w_ext[T0 : T + 1, :])
            nc.scalar.dma_start(onesb[:], onesb_ext[:])

            GD = 4  # max samples per group (psum rows 0/32/64/96)
            SW = 2 * D  # free width per sample in x/t/e/p group tiles
            # small groups first (ACT starts before the DMA pipeline fills),
            # GD=2 tail groups to shorten the drain
            sizes = [4] * 16
            assert sum(sizes) == ns
            base = 0
            for gi, gs in enumerate(sizes):
                early = gi < 1
                last = gi == len(sizes) - 1
                xg = xpool.tile([T0, GD * SW], F32R, tag="xg")
                for j in range(gs):
                    if early:
                        nc.sync.dma_start(
                            xg[0:64, j * SW : j * SW + D],
                            x_ext[base + j, 0:64, :],
                        )
                        nc.scalar.dma_start(
                            xg[64:T0, j * SW : j * SW + D],
                            x_ext[base + j, 64:T0, :],
                        )
                        nc.gpsimd.dma_start(
                            xg[0:36, j * SW + D : (j + 1) * SW],
                            x_ext[base + j, T0 : T0 + 36, :],
                        )
                        nc.gpsimd.dma_start(
                            xg[36:T1E, j * SW + D : (j + 1) * SW],
                            x_ext[base + j, T0 + 36 : T + 1, :],
                        )
                    else:
                        nc.sync.dma_start(
                            xg[:, j * SW : j * SW + D], x_ext[base + j, 0:T0, :]
                        )
                        nc.gpsimd.dma_start(
                            xg[0:T1E, j * SW + D : (j + 1) * SW],
                            x_ext[base + j, T0 : T + 1, :],
                        )

                tg = tpool.tile([T0, GD * SW], F32, tag="tg")
                eg = epool.tile([T0, GD * SW], BF16, tag="eg")

                for j in range(gs):
                    xt0 = xg[:, j * SW : j * SW + D]
                    xt1 = xg[0:T1E, j * SW + D : (j + 1) * SW]
                    # scores: s0-half [128, 512] and s1-half [72, 512] of one
                    # two-bank psum tile
                    ps = ppool.tile([T0, 2 * D], F32, tag="ps")
                    nc.tensor.matmul(
                        out=ps[:, 0:D], lhsT=w0[:, 0:T0], rhs=xt0,
                        start=True, stop=False,
                    )
                    nc.tensor.matmul(
                        out=ps[:, 0:D], lhsT=w1[:, 0:T0], rhs=xt1,
                        start=False, stop=True,
                    )
                    nc.tensor.matmul(
                        out=ps[0:T1, D : 2 * D], lhsT=w0[:, T0:T], rhs=xt0,
                        start=True, stop=False,
                    )
                    nc.tensor.matmul(
                        out=ps[0:T1, D : 2 * D], lhsT=w1[:, T0:T], rhs=xt1,
                        start=False, stop=True,
                    )

                    # tanh over the whole sample (bias already folded in)
                    nc.scalar.activation(
                        out=tg[:, j * SW : (j + 1) * SW], in_=ps[:],
                        func=mybir.ActivationFunctionType.Tanh,
                    )

                # exp (bf16 out): halves mid-kernel, quarters in the last group
                nexp = 4 if last else 2
                nexp = min(nexp, gs * 2)
                H = gs * SW // nexp
                for h in range(nexp):
                    nc.scalar.activation(
                        out=eg[:, h * H : (h + 1) * H],
                        in_=tg[:, h * H : (h + 1) * H],
                        func=mybir.ActivationFunctionType.Exp,
                    )

                # P = E * X (bf16 out), half-group ops
                pg = epool.tile([T0, GD * SW], BF16, tag="pg")
                nmul = 2 if gs > 1 else 1
                M = gs * SW // nmul
                for h in range(nmul):
                    nc.vector.tensor_mul(
                        pg[:, h * M : (h + 1) * M],
                        eg[:, h * M : (h + 1) * M],
                        xg[:, h * M : (h + 1) * M].bitcast(F32),
                    )

                # numer rows at partitions {0,32,64,96} of R1, denoms in R2;
                # emission interleaved across PE column groups
                r1 = rpool.tile([128, D], F32, tag="r1")
                r2 = rpool.tile([128, D], F32, tag="r2")
                for rt, srct in ((r1, pg), (r2, eg)):
                    for cs, kk, st in ((0, T0, True), (D, T1, False)):
                        for k in range(gs):
                            nc.tensor.matmul(
                                out=rt[32 * k : 32 * k + 1, :],
                                lhsT=onesb[0:kk, :],
                                rhs=srct[0:kk, k * SW + cs : k * SW + cs + D],
                                start=st, stop=not st,
                                tile_position=(0, 32 * k),
                            )

                # batched epilogue over rows {0,32,...} (between-rows lanes
                # hold garbage; only the gs real rows are DMA'd out)
                nrows = 32 * (gs - 1) + 1
                rcp = apool.tile([nrows, D], F32, tag="rcp")
                nc.vector.reciprocal_approx_fast(rcp[:], r2[0:nrows, :])
                og = apool.tile([nrows, D], F32, tag="og")
                nc.vector.tensor_mul(og[:], r1[0:nrows, :], rcp[:])
                nc.sync.dma_start(
                    out_ext[base : base + gs, :], og[0:nrows:32, :],
                )
                base += gs

    nc.compile()
    return nc


def _get_nc(ns=NS):
    if ns not in _CACHE:
        _CACHE[ns] = _build(ns)
    return _CACHE[ns]


def _run(inputs, W, b, trace=False, **trace_kw):
    x0 = np.asarray(inputs, dtype=np.float32)
    x = np.empty((B, T + 1, D), dtype=np.float32)
    x[:, 0:T, :] = x0
    x[:, T, :] = 1.0
    w = np.asarray(W, dtype=np.float32)
    bv = np.asarray(b, dtype=np.float32)
    wext = np.ascontiguousarray(np.concatenate([w, bv[None, :]], axis=0))
    onesb = np.ones((T0, 1), dtype=ml_dtypes.bfloat16)

    nc = _get_nc()
    in_maps = [
        {
            "x": np.ascontiguousarray(x[c * NS : (c + 1) * NS]),
            "w": wext,
            "onesb": onesb,
        }
        for c in range(N_CORES)
    ]
    res = run_bass_kernel_spmd(
        nc, in_maps, core_ids=list(range(N_CORES)), trace=trace, **trace_kw
    )
    out = np.concatenate([res.results[c]["out"] for c in range(N_CORES)], axis=0)
    return out, res


def kernel(**inputs) -> np.ndarray:
    out, _ = _run(inputs["inputs"], inputs["W"], inputs["b"])
    return out


# revision 57
# speedup vs baseline: 1.2633x; 1.2633x over previous
"""Trainium2 Bass kernel for nn_AttentionLayer (attention pooling).

Reference computation (per sample b):
    scores[s, d] = tanh( sum_t X[t, d] * W[t, s] + bias[s] )   # X = inputs[b], [T=200, D=512]
    a = softmax over s of scores                                # per d column
    out[b, d] = sum_s a[s, d] * X[s, d]

Sharding: pure data parallel, batch 512 -> 64 samples on each of 8 cores.
W/b replicated. No collectives.

Layout (per sample): s on partitions (chunks 128+72), d on free axis; the
bias is folded into the score matmul via W_ext = [W; b] (host-side) and a
memset ones-row appended to X's second t-chunk. Each sample's scores live
in one [128, 1024] PSUM tile (s0-half cols 0:512 on 128 partitions,
s1-half cols 512:1024 on 72 partitions; unused lanes carry garbage that
tanh bounds and the reductions mask).

  - 4 float32r matmuls per sample accumulate scores into PSUM
  - one ACT tanh [128, 1024] per sample; exp in half-group chunks (bf16)
  - one DVE multiply P = E * X per 4-sample group [128, 4096]
  - bf16 ones-vector matmuls (col-tiled, 4 PE column groups) reduce over s
    into numer/denom rows at partitions {0,32,64,96}
  - batched DVE reciprocal+multiply epilogue, strided-row output DMA
"""

import sys

if "/opt/trn_rl_repo" not in sys.path:
    sys.path.insert(0, "/opt/trn_rl_repo")

import ml_dtypes
import numpy as np

import concourse.bass as bass
import concourse.tile as tile
from concourse import bacc, mybir
from concourse.bass_utils import run_bass_kernel_spmd

B, T, D = 512, 200, 512
N_CORES = 8
NS = B // N_CORES  # samples per core
T0 = 128           # first t/s chunk
T1 = T - T0        # 72
T1E = T1 + 1       # 73: second t-chunk + bias ones-row

F32 = mybir.dt.float32
F32R = mybir.dt.float32r
BF16 = mybir.dt.bfloat16

_CACHE = {}


def _build(ns=NS):
    nc = bacc.Bacc("TRN2", target_bir_lowering=False, debug=False)

    x_ext = nc.declare_dram_parameter("x", [ns, T, D], F32R, isOutput=False)
    w_ext = nc.declare_dram_parameter("w", [T + 1, T], F32R, isOutput=False)
    onesb_ext = nc.declare_dram_parameter("onesb", [T0, 1], BF16, isOutput=False)
    onesrow_ext = nc.declare_dram_parameter("onesrow", [4, D], F32R, isOutput=False)
    out_ext = nc.declare_dram_parameter("out", [ns, D], F32, isOutput=True)

    with tile.TileContext(nc) as tc:
        with (
            tc.tile_pool(name="const", bufs=1) as cpool,
            tc.tile_pool(name="tanh", bufs=2) as tpool,
            tc.tile_pool(name="exp", bufs=3) as epool,
            tc.tile_pool(name="small", bufs=3) as apool,
            tc.tile_pool(name="xin", bufs=4) as xpool,
            tc.tile_pool(name="psum", bufs=2, space="PSUM") as ppool,
            tc.tile_pool(name="rpsum", bufs=2, space="PSUM") as rpool,
        ):
            # constants: extended weights ([W; b]), ones vector
            w0 = cpool.tile([T0, T], F32R)    # W[0:128, :]  (t on part, s free)
            w1 = cpool.tile([T1E, T], F32R)   # [W[128:200, :]; b]
            onesb = cpool.tile([T0, 1], BF16)


            nc.scalar.dma_start(w0[:], w_ext[0:T0, :])
            nc.scalar.dma_start(w1[:], w_ext[T0 : T + 1, :])
            nc.scalar.dma_start(onesb[:], onesb_ext[:])

            GD = 4  # max samples per group (psum rows 0/32/64/96)
            SW = 2 * D  # free width per sample in x/t/e/p group tiles
            # small groups first (ACT starts before the DMA pipeline fills),
            # GD=2 tail groups to shorten the drain
            sizes = [4] * 16
            assert sum(sizes) == ns
            base = 0
            for gi, gs in enumerate(sizes):
                early = gi < 1
                last = gi == len(sizes) - 1
                xg = xpool.tile([T0, GD * SW], F32R, tag="xg")
                for j in range(gs):
                    if early:
                        nc.sync.dma_start(
                            xg[0:64, j * SW : j * SW + D],
                            x_ext[base + j, 0:64, :],
                        )
                        nc.scalar.dma_start(
                            xg[64:T0, j * SW : j * SW + D],
                            x_ext[base + j, 64:T0, :],
                        )
                        nc.gpsimd.dma_start(
                            xg[0:36, j * SW + D : (j + 1) * SW],
                            x_ext[base + j, T0 : T0 + 36, :],
                        )
                        nc.gpsimd.dma_start(
                            xg[36:T1, j * SW + D : (j + 1) * SW],
                            x_ext[base + j, T0 + 36 : T, :],
                        )
                    else:
                        nc.sync.dma_start(
                            xg[:, j * SW : j * SW + D], x_ext[base + j, 0:T0, :]
                        )
                        nc.gpsimd.dma_start(
                            xg[0:T1, j * SW + D : (j + 1) * SW],
                            x_ext[base + j, T0:T, :],
                        )
                # ones rows for the group in one strided DMA
                nc.gpsimd.dma_start(
                    xg[T1 : T1 + 1, 0 : gs * SW].rearrange(
                        "p (j w) -> p j w", j=gs
                    )[:, :, D : 2 * D],
                    onesrow_ext[0:gs, :].rearrange("j d -> () j d"),
                )

                tg = tpool.tile([T0, GD * SW], F32, tag="tg")
                eg = epool.tile([T0, GD * SW], BF16, tag="eg")

                for j in range(gs):
                    xt0 = xg[:, j * SW : j * SW + D]
                    xt1 = xg[0:T1E, j * SW + D : (j + 1) * SW]
                    # scores: s0-half [128, 512] and s1-half [72, 512] of one
                    # two-bank psum tile
                    ps = ppool.tile([T0, 2 * D], F32, tag="ps")
                    nc.tensor.matmul(
                        out=ps[:, 0:D], lhsT=w0[:, 0:T0], rhs=xt0,
                        start=True, stop=False,
                    )
                    nc.tensor.matmul(
                        out=ps[:, 0:D], lhsT=w1[:, 0:T0], rhs=xt1,
                        start=False, stop=True,
                    )
                    nc.tensor.matmul(
                        out=ps[0:T1, D : 2 * D], lhsT=w0[:, T0:T], rhs=xt0,
                        start=True, stop=False,
                    )
                    nc.tensor.matmul(
                        out=ps[0:T1, D : 2 * D], lhsT=w1[:, T0:T], rhs=xt1,
                        start=False, stop=True,
                    )

                    # tanh over the whole sample (bias already folded in)
                    nc.scalar.activation(
                        out=tg[:, j * SW : (j + 1) * SW], in_=ps[:],
                        func=mybir.ActivationFunctionType.Tanh,
                    )

                # exp (bf16 out): halves mid-kernel, quarters in the last group
                nexp = 4 if last else 2
                nexp = min(nexp, gs * 2)
                H = gs * SW // nexp
                for h in range(nexp):
                    nc.scalar.activation(
                        out=eg[:, h * H : (h + 1) * H],
                        in_=tg[:, h * H : (h + 1) * H],
                        func=mybir.ActivationFunctionType.Exp,
                    )

                # P = E * X (bf16 out), half-group ops
                pg = epool.tile([T0, GD * SW], BF16, tag="pg")
                nmul = 2 if gs > 1 else 1
                M = gs * SW // nmul
                for h in range(nmul):
                    nc.vector.tensor_mul(
                        pg[:, h * M : (h + 1) * M],
                        eg[:, h * M : (h + 1) * M],
                        xg[:, h * M : (h + 1) * M].bitcast(F32),
                    )

                # numer rows at partitions {0,32,64,96} of R1, denoms in R2;
                # emission interleaved across PE column groups
                r1 = rpool.tile([128, D], F32, tag="r1")
                r2 = rpool.tile([128, D], F32, tag="r2")
                for rt, srct in ((r1, pg), (r2, eg)):
                    for cs, kk, st in ((0, T0, True), (D, T1, False)):
                        for k in range(gs):
                            nc.tensor.matmul(
                                out=rt[32 * k : 32 * k + 1, :],
                                lhsT=onesb[0:kk, :],
                                rhs=srct[0:kk, k * SW + cs : k * SW + cs + D],
                                start=st, stop=not st,
                                tile_position=(0, 32 * k),
                            )

                # batched epilogue over rows {0,32,...} (between-rows lanes
                # hold garbage; only the gs real rows are DMA'd out)
                nrows = 32 * (gs - 1) + 1
                rcp = apool.tile([nrows, D], F32, tag="rcp")
                nc.vector.reciprocal_approx_fast(rcp[:], r2[0:nrows, :])
                og = apool.tile([nrows, D], F32, tag="og")
                nc.vector.tensor_mul(og[:], r1[0:nrows, :], rcp[:])
                nc.sync.dma_start(
                    out_ext[base : base + gs, :], og[0:nrows:32, :],
                )
                base += gs

    nc.compile()
    return nc


def _get_nc(ns=NS):
    if ns not in _CACHE:
        _CACHE[ns] = _build(ns)
    return _CACHE[ns]


def _run(inputs, W, b, trace=False, **trace_kw):
    x = np.ascontiguousarray(np.asarray(inputs, dtype=np.float32))
    w = np.asarray(W, dtype=np.float32)
    bv = np.asarray(b, dtype=np.float32)
    wext = np.ascontiguousarray(np.concatenate([w, bv[None, :]], axis=0))
    onesb = np.ones((T0, 1), dtype=ml_dtypes.bfloat16)
    onesrow = np.ones((4, D), dtype=np.float32)

    nc = _get_nc()
    in_maps = [
        {
            "x": np.ascontiguousarray(x[c * NS : (c + 1) * NS]),
            "w": wext,
            "onesb": onesb,
            "onesrow": onesrow,
        }
        for c in range(N_CORES)
    ]
    res = run_bass_kernel_spmd(
        nc, in_maps, core_ids=list(range(N_CORES)), trace=trace, **trace_kw
    )
    out = np.concatenate([res.results[c]["out"] for c in range(N_CORES)], axis=0)
    return out, res


def kernel(**inputs) -> np.ndarray:
    out, _ = _run(inputs["inputs"], inputs["W"], inputs["b"])
    return out


# revision 58
# speedup vs baseline: 1.2690x; 1.0045x over previous
"""Trainium2 Bass kernel for nn_AttentionLayer (attention pooling).

Reference computation (per sample b):
    scores[s, d] = tanh( sum_t X[t, d] * W[t, s] + bias[s] )   # X = inputs[b], [T=200, D=512]
    a = softmax over s of scores                                # per d column
    out[b, d] = sum_s a[s, d] * X[s, d]

Sharding: pure data parallel, batch 512 -> 64 samples on each of 8 cores.
W/b replicated. No collectives.

Layout (per sample): s on partitions (chunks 128+72), d on free axis; the
bias is folded into the score matmul via W_ext = [W; b] (host-side) and a
memset ones-row appended to X's second t-chunk. Each sample's scores live
in one [128, 1024] PSUM tile (s0-half cols 0:512 on 128 partitions,
s1-half cols 512:1024 on 72 partitions; unused lanes carry garbage that
tanh bounds and the reductions mask).

  - 4 float32r matmuls per sample accumulate scores into PSUM
  - one ACT tanh [128, 1024] per sample; exp in half-group chunks (bf16)
  - one DVE multiply P = E * X per 4-sample group [128, 4096]
  - bf16 ones-vector matmuls (col-tiled, 4 PE column groups) reduce over s
    into numer/denom rows at partitions {0,32,64,96}
  - batched DVE reciprocal+multiply epilogue, strided-row output DMA
"""

import sys

if "/opt/trn_rl_repo" not in sys.path:
    sys.path.insert(0, "/opt/trn_rl_repo")

import ml_dtypes
import numpy as np

import concourse.bass as bass
import concourse.tile as tile
from concourse import bacc, mybir
from concourse.bass_utils import run_bass_kernel_spmd

B, T, D = 512, 200, 512
N_CORES = 8
NS = B // N_CORES  # samples per core
T0 = 128           # first t/s chunk
T1 = T - T0        # 72
T1E = T1 + 1       # 73: second t-chunk + bias ones-row

F32 = mybir.dt.float32
F32R = mybir.dt.float32r
BF16 = mybir.dt.bfloat16

_CACHE = {}


def _build(ns=NS):
    nc = bacc.Bacc("TRN2", target_bir_lowering=False, debug=False)

    x_ext = nc.declare_dram_parameter("x", [ns, T, D], F32R, isOutput=False)
    w_ext = nc.declare_dram_parameter("w", [T + 1, T], F32R, isOutput=False)
    onesb_ext = nc.declare_dram_parameter("onesb", [T0, 1], BF16, isOutput=False)
    onesrow_ext = nc.declare_dram_parameter("onesrow", [4, D], F32R, isOutput=False)
    out_ext = nc.declare_dram_parameter("out", [ns, D], F32, isOutput=True)

    with tile.TileContext(nc) as tc:
        with (
            tc.tile_pool(name="const", bufs=1) as cpool,
            tc.tile_pool(name="tanh", bufs=2) as tpool,
            tc.tile_pool(name="exp", bufs=3) as epool,
            tc.tile_pool(name="small", bufs=3) as apool,
            tc.tile_pool(name="xin", bufs=5) as xpool,
            tc.tile_pool(name="psum", bufs=2, space="PSUM") as ppool,
            tc.tile_pool(name="rpsum", bufs=2, space="PSUM") as rpool,
        ):
            # constants: extended weights ([W; b]), ones vector
            w0 = cpool.tile([T0, T], F32R)    # W[0:128, :]  (t on part, s free)
            w1 = cpool.tile([T1E, T], F32R)   # [W[128:200, :]; b]
            onesb = cpool.tile([T0, 1], BF16)


            nc.scalar.dma_start(w0[:], w_ext[0:T0, :])
            nc.scalar.dma_start(w1[:], w_ext[T0 : T + 1, :])
            nc.scalar.dma_start(onesb[:], onesb_ext[:])

            GD = 4  # max samples per group (psum rows 0/32/64/96)
            SW = 2 * D  # free width per sample in x/t/e/p group tiles
            # small groups first (ACT starts before the DMA pipeline fills),
            # GD=2 tail groups to shorten the drain
            sizes = [4] * 16
            assert sum(sizes) == ns
            base = 0
            for gi, gs in enumerate(sizes):
                early = gi < 1
                last = gi == len(sizes) - 1
                xg = xpool.tile([T0, GD * SW], F32R, tag="xg")
                for j in range(gs):
                    if early:
                        nc.sync.dma_start(
                            xg[0:64, j * SW : j * SW + D],
                            x_ext[base + j, 0:64, :],
                        )
                        nc.scalar.dma_start(
                            xg[64:T0, j * SW : j * SW + D],
                            x_ext[base + j, 64:T0, :],
                        )
                        nc.gpsimd.dma_start(
                            xg[0:36, j * SW + D : (j + 1) * SW],
                            x_ext[base + j, T0 : T0 + 36, :],
                        )
                        nc.gpsimd.dma_start(
                            xg[36:T1, j * SW + D : (j + 1) * SW],
                            x_ext[base + j, T0 + 36 : T, :],
                        )
                    else:
                        nc.sync.dma_start(
                            xg[:, j * SW : j * SW + D], x_ext[base + j, 0:T0, :]
                        )
                        nc.gpsimd.dma_start(
                            xg[0:T1, j * SW + D : (j + 1) * SW],
                            x_ext[base + j, T0:T, :],
                        )
                # ones rows for the group in one strided DMA
                nc.gpsimd.dma_start(
                    xg[T1 : T1 + 1, 0 : gs * SW].rearrange(
                        "p (j w) -> p j w", j=gs
                    )[:, :, D : 2 * D],
                    onesrow_ext[0:gs, :].rearrange("j d -> () j d"),
                )

                tg = tpool.tile([T0, GD * SW], F32, tag="tg")
                eg = epool.tile([T0, GD * SW], BF16, tag="eg")

                for j in range(gs):
                    xt0 = xg[:, j * SW : j * SW + D]
                    xt1 = xg[0:T1E, j * SW + D : (j + 1) * SW]
                    # scores: s0-half [128, 512] and s1-half [72, 512] of one
                    # two-bank psum tile
                    ps = ppool.tile([T0, 2 * D], F32, tag="ps")
                    nc.tensor.matmul(
                        out=ps[:, 0:D], lhsT=w0[:, 0:T0], rhs=xt0,
                        start=True, stop=False,
                    )
                    nc.tensor.matmul(
                        out=ps[:, 0:D], lhsT=w1[:, 0:T0], rhs=xt1,
                        start=False, stop=True,
                    )
                    nc.tensor.matmul(
                        out=ps[0:T1, D : 2 * D], lhsT=w0[:, T0:T], rhs=xt0,
                        start=True, stop=False,
                    )
                    nc.tensor.matmul(
                        out=ps[0:T1, D : 2 * D], lhsT=w1[:, T0:T], rhs=xt1,
                        start=False, stop=True,
                    )

                    # tanh over the whole sample (bias already folded in)
                    nc.scalar.activation(
                        out=tg[:, j * SW : (j + 1) * SW], in_=ps[:],
                        func=mybir.ActivationFunctionType.Tanh,
                    )

                # exp (bf16 out): halves mid-kernel, quarters in the last group
                nexp = 4 if last else 2
                nexp = min(nexp, gs * 2)
                H = gs * SW // nexp
                for h in range(nexp):
                    nc.scalar.activation(
                        out=eg[:, h * H : (h + 1) * H],
                        in_=tg[:, h * H : (h + 1) * H],
                        func=mybir.ActivationFunctionType.Exp,
                    )

                # P = E * X (bf16 out), half-group ops
                pg = epool.tile([T0, GD * SW], BF16, tag="pg")
                nmul = 2 if gs > 1 else 1
                M = gs * SW // nmul
                for h in range(nmul):
                    nc.vector.tensor_mul(
                        pg[:, h * M : (h + 1) * M],
                        eg[:, h * M : (h + 1) * M],
                        xg[:, h * M : (h + 1) * M].bitcast(F32),
                    )

                # numer rows at partitions {0,32,64,96} of R1, denoms in R2;
                # emission interleaved across PE column groups
                r1 = rpool.tile([128, D], F32, tag="r1")
                r2 = rpool.tile([128, D], F32, tag="r2")
                for rt, srct in ((r1, pg), (r2, eg)):
                    for cs, kk, st in ((0, T0, True), (D, T1, False)):
                        for k in range(gs):
                            nc.tensor.matmul(
                                out=rt[32 * k : 32 * k + 1, :],
                                lhsT=onesb[0:kk, :],
                                rhs=srct[0:kk, k * SW + cs : k * SW + cs + D],
                                start=st, stop=not st,
                                tile_position=(0, 32 * k),
                            )

                # batched epilogue over rows {0,32,...} (between-rows lanes
                # hold garbage; only the gs real rows are DMA'd out)
                nrows = 32 * (gs - 1) + 1
                rcp = apool.tile([nrows, D], F32, tag="rcp")
                nc.vector.reciprocal_approx_fast(rcp[:], r2[0:nrows, :])
                og = apool.tile([nrows, D], F32, tag="og")
                nc.vector.tensor_mul(og[:], r1[0:nrows, :], rcp[:])
                nc.sync.dma_start(
                    out_ext[base : base + gs, :], og[0:nrows:32, :],
                )
                base += gs

    nc.compile()
    return nc


def _get_nc(ns=NS):
    if ns not in _CACHE:
        _CACHE[ns] = _build(ns)
    return _CACHE[ns]


def _run(inputs, W, b, trace=False, **trace_kw):
    x = np.ascontiguousarray(np.asarray(inputs, dtype=np.float32))
    w = np.asarray(W, dtype=np.float32)
    bv = np.asarray(b, dtype=np.float32)
    wext = np.ascontiguousarray(np.concatenate([w, bv[None, :]], axis=0))
    onesb = np.ones((T0, 1), dtype=ml_dtypes.bfloat16)
    onesrow = np.ones((4, D), dtype=np.float32)

    nc = _get_nc()
    in_maps = [
        {
            "x": np.ascontiguousarray(x[c * NS : (c + 1) * NS]),
            "w": wext,
            "onesb": onesb,
            "onesrow": onesrow,
        }
        for c in range(N_CORES)
    ]
    res = run_bass_kernel_spmd(
        nc, in_maps, core_ids=list(range(N_CORES)), trace=trace, **trace_kw
    )
    out = np.concatenate([res.results[c]["out"] for c in range(N_CORES)], axis=0)
    return out, res


def kernel(**inputs) -> np.ndarray:
    out, _ = _run(inputs["inputs"], inputs["W"], inputs["b"])
    return out
